# revision 10
# baseline (speedup 1.0000x reference)
"""Trainium2 Bass kernel for the DescriptorLoss dual-softmax loss.

Math (per batch element b):
    des1 = p1[b][:, y1, x1]            # [C=256, N=3540]
    des2 = p2[b][:, y2, x2]            # [C, N]
    dist = TEMP * des1.T @ des2        # [N, N]
    loss_b = 2*trace(dist) - sum_m lse_row[m] - sum_n lse_col[n]
    loss   = -(sum_b loss_b) / (B*N)

Key identities used on-device:
    trace(dist)   = TEMP * <des1, des2>_Frobenius  (elementwise, no matmul)
    lse (no max-subtraction) is safe: |dist| <~ 20, exp fits fp32/bf16 range.

Sharding: data-parallel over the batch dim, one batch element per
NeuronCore (B == 8 == n_cores).  The host gathers descriptors with the
int32 index arrays (pure data movement), casts to bf16, runs the SPMD
program, and averages the 8 per-core partial sums.

Per-core engine assignment:
    PE : dist tiles (bf16 inputs, fp32 PSUM accumulate over C=2x128)
         + ones-matmul partition reductions at the end
    ACT: exp(TEMP*dist) PSUM->SBUF(bf16), accum_out = row sums (free!)
         + final Ln (with accum_out again for the sum of logs)
    DVE: column-sum accumulation in bf16 (2x mode) + diag term via
         scalar_tensor_tensor's accum_out

Pipeline: each m-tile's 3540 dist columns live in two PSUM regions
(1536 + 2004 fp32 = 3 + 4 banks).  PE refills a region only after ACT
finished exp-ing it (WAR tracked by Tile at byte granularity); region
sizes are chosen so each refill fits inside ACT's work on the other
region, keeping ACT ~100% busy (the kernel is ACT-bound).
"""

import numpy as np
import ml_dtypes

B = 8
C = 256
N = 3540
TEMP = 0.2
KP = 128           # contraction chunk (partition dim)
NK = C // KP       # 2
MT = 128           # rows per m-tile
N_MTILES = (N + MT - 1) // MT   # 28 (last tile has 84 rows)
REGIONS = [(0, 1536), (1536, N - 1536)]   # 3 + 4 PSUM banks
MM_N = 512         # max moving free dim per matmul
D1_SPLIT = 512     # head columns of des1 loaded first (m-tiles 0..3)

_prog_cache = {}


def _mm_chunks(width):
    out = []
    off = 0
    while off < width:
        w = min(MM_N, width - off)
        out.append((off, w))
        off += w
    return out


def _build_program():
    import concourse.bacc as bacc
    import concourse.tile as tile
    from concourse import mybir

    dt = mybir.dt
    f32 = dt.float32
    bf16 = dt.bfloat16
    Exp = mybir.ActivationFunctionType.Exp
    Ln = mybir.ActivationFunctionType.Ln
    ADD = mybir.AluOpType.add
    MULT = mybir.AluOpType.mult

    nc = bacc.Bacc(
        "TRN2", target_bir_lowering=False, debug=False, num_devices=B)
    d1 = nc.dram_tensor("d1", [C, N], bf16, kind="ExternalInput")
    d2 = nc.dram_tensor("d2", [C, N], bf16, kind="ExternalInput")
    out = nc.dram_tensor("out", [1, 3], f32, kind="ExternalOutput")

    with tile.TileContext(nc) as tc:
        with (
            tc.tile_pool(name="persist", bufs=1) as persist,
            tc.tile_pool(name="etiles", bufs=2) as etiles,
            tc.tile_pool(name="small", bufs=1) as small,
        ):
            # ---- load descriptors (bf16, [128, N] per C-chunk) ----
            # Split + ordered so the first m-tiles' operands land first,
            # spread over two HWDGE queues (sync + vector).
            d1_sb = [persist.tile([KP, N], bf16, tag=f"d1_{k}", name=f"d1_{k}")
                     for k in range(NK)]
            d2_sb = [persist.tile([KP, N], bf16, tag=f"d2_{k}", name=f"d2_{k}")
                     for k in range(NK)]
            g0, w0 = REGIONS[0]
            g1, w1 = REGIONS[1]
            for k in range(NK):   # des1 head columns: lhsT for m-tiles 0..3
                nc.sync.dma_start(out=d1_sb[k][:, 0:D1_SPLIT],
                                  in_=d1[k * KP:(k + 1) * KP, 0:D1_SPLIT])
            for k in range(NK):   # des2 region 0 (scalar queue)
                nc.scalar.dma_start(out=d2_sb[k][:, g0:g0 + w0],
                                    in_=d2[k * KP:(k + 1) * KP, g0:g0 + w0])
            for k in range(NK):   # des2 region 1 (sync queue)
                nc.sync.dma_start(out=d2_sb[k][:, g1:g1 + w1],
                                  in_=d2[k * KP:(k + 1) * KP, g1:g1 + w1])
            for k in range(NK):   # rest of des1
                nc.scalar.dma_start(out=d1_sb[k][:, D1_SPLIT:N],
                                    in_=d1[k * KP:(k + 1) * KP, D1_SPLIT:N])

            colacc = persist.tile([MT, N], bf16, tag="colacc", name="colacc")
            nc.vector.memset(colacc, 0.0)

            # rsparts[:, r*N_MTILES + t] = rowsum of exp over region r of
            # m-tile t.  0.5-init: rows of the last (84-row) m-tile that do
            # not exist end up 0.5+0.5 = 1.0 -> Ln contributes 0.
            rsparts = small.tile([MT, 2 * N_MTILES], f32, tag="rsparts",
                                 name="rsparts")
            nc.vector.memset(rsparts, 0.5)

            ones_bf = small.tile([KP, 1], bf16, name="ones_bf")
            nc.vector.memset(ones_bf, 1.0)
            ones_f32 = small.tile([KP, 1], f32, name="ones_f32")
            nc.vector.memset(ones_f32, 1.0)

            # fin[:,0] = diag partial, fin[:,1] = sum of row-logs partial
            fin = small.tile([KP, 2], f32, tag="fin", name="fin")

            # ---- diag term: sum(des1 * des2) per partition ----
            scratch = persist.tile([KP, N], bf16, tag="scratch", name="scratch")
            diag0 = small.tile([KP, 1], f32, name="diag0")
            diag1 = small.tile([KP, 1], f32, name="diag1")
            nc.vector.scalar_tensor_tensor(
                out=scratch, in0=d1_sb[0], scalar=1.0, in1=d2_sb[0],
                op0=MULT, op1=MULT, accum_out=diag0)
            nc.vector.scalar_tensor_tensor(
                out=scratch, in0=d1_sb[1], scalar=1.0, in1=d2_sb[1],
                op0=MULT, op1=MULT, accum_out=diag1)
            nc.vector.tensor_add(fin[:, 0:1], diag0, diag1)

            # ---- main loop over m-tiles ----
            import contextlib
            with contextlib.ExitStack() as psctx:
                psA = psctx.enter_context(
                    tc.tile_pool(name="psA", bufs=1, space="PSUM"))
                psB = psctx.enter_context(
                    tc.tile_pool(name="psB", bufs=1, space="PSUM"))
                pspools = [psA, psB]
                pstags = ["psA", "psB"]
                for t in range(N_MTILES):
                    m0 = t * MT
                    mp = min(MT, N - m0)
                    ps = [pspools[r].tile([MT, REGIONS[r][1]], f32,
                                          tag=pstags[r], name=f"ps{r}")
                          for r in range(2)]
                    # Region-outer, k-inner: frees region 0 early so the
                    # next tile's refill hides under ACT's region-1 work.
                    for r in range(2):
                        g, gw = REGIONS[r]
                        for k in range(NK):
                            for (off, w) in _mm_chunks(gw):
                                nc.tensor.matmul(
                                    ps[r][:mp, off:off + w],
                                    lhsT=d1_sb[k][:, m0:m0 + mp],
                                    rhs=d2_sb[k][:, g + off:g + off + w],
                                    start=(k == 0), stop=(k == NK - 1))
                    for r in range(2):
                        g, gw = REGIONS[r]
                        e = etiles.tile([MT, gw], bf16, tag=f"e{r}",
                                        name=f"e{r}")
                        nc.scalar.activation(
                            out=e[:mp, :], in_=ps[r][:mp, :], func=Exp,
                            scale=TEMP,
                            accum_out=rsparts[:mp, r * N_MTILES + t:
                                              r * N_MTILES + t + 1])
                        nc.vector.tensor_add(
                            colacc[:mp, g:g + gw],
                            colacc[:mp, g:g + gw],
                            e[:mp, :])

            # ---- finalize ----
            # rowsums; invalid rows = 1.0 -> Ln 0.  Ln's accum_out gives the
            # per-partition sum of logs directly.
            rowsums = small.tile([MT, N_MTILES], f32, tag="rowsums",
                                 name="rowsums")
            nc.vector.tensor_add(
                rowsums, rsparts[:, 0:N_MTILES],
                rsparts[:, N_MTILES:2 * N_MTILES])
            rl = small.tile([MT, N_MTILES], f32, tag="rl", name="rl")
            nc.scalar.activation(out=rl, in_=rowsums, func=Ln,
                                 accum_out=fin[:, 1:2])

            with tc.tile_pool(name="psF", bufs=1, space="PSUM") as psF:
                # column sums: ones-matmuls into one 7-bank PSUM strip,
                # then a single Ln whose accum_out is sum(log(colsum)).
                csum = psF.tile([1, 3584], f32, tag="csum", name="csum")
                for (off, w) in _mm_chunks(N):
                    nc.tensor.matmul(csum[0:1, off:off + w], lhsT=ones_bf,
                                     rhs=colacc[:, off:off + w],
                                     start=True, stop=True)
                cl = small.tile([1, N], f32, tag="cl", name="cl")
                clsum = small.tile([1, 1], f32, tag="clsum", name="clsum")
                nc.scalar.activation(out=cl, in_=csum[0:1, 0:N], func=Ln,
                                     accum_out=clsum)

                # partition-reduce diag and row-log partials in one matmul
                dr_ps = psF.tile([1, 2], f32, tag="drps", name="dr_ps")
                nc.tensor.matmul(dr_ps[0:1, 0:2], lhsT=ones_f32,
                                 rhs=fin[:, 0:2], start=True, stop=True)

                outsb = small.tile([1, 3], f32, tag="outsb", name="outsb")
                nc.vector.tensor_copy(outsb[0:1, 0:2], dr_ps[0:1, 0:2])
                nc.vector.tensor_copy(outsb[0:1, 2:3], clsum)
                nc.sync.dma_start(out=out[:, :], in_=outsb)

    nc.compile()
    return nc


def _get_program():
    if "nc" not in _prog_cache:
        _prog_cache["nc"] = _build_program()
    return _prog_cache["nc"]


def kernel(**inputs) -> np.ndarray:
    from concourse.bass_utils import run_bass_kernel_spmd

    p1 = np.asarray(inputs["p1"], dtype=np.float32)
    p2 = np.asarray(inputs["p2"], dtype=np.float32)
    y1 = np.asarray(inputs["y1"]).astype(np.int64)
    x1 = np.asarray(inputs["x1"]).astype(np.int64)
    y2 = np.asarray(inputs["y2"]).astype(np.int64)
    x2 = np.asarray(inputs["x2"]).astype(np.int64)

    # Host-side gather (data movement only): [B, C, N] then bf16 cast.
    des1 = p1[:, :, y1, x1].astype(ml_dtypes.bfloat16)
    des2 = p2[:, :, y2, x2].astype(ml_dtypes.bfloat16)

    nc = _get_program()
    in_maps = [
        {"d1": np.ascontiguousarray(des1[b]), "d2": np.ascontiguousarray(des2[b])}
        for b in range(B)
    ]
    res = run_bass_kernel_spmd(nc, in_maps, list(range(B)))
    total = 0.0
    for b in range(B):
        d, r, c = (float(v) for v in np.asarray(res.results[b]["out"]).ravel())
        total += 2.0 * TEMP * d - r - c
    loss = -total / (B * N)
    return np.float32(loss)


# revision 11
# speedup vs baseline: 1.0212x; 1.0212x over previous
"""Trainium2 Bass kernel for the DescriptorLoss dual-softmax loss.

Math (per batch element b):
    des1 = p1[b][:, y1, x1]            # [C=256, N=3540]
    des2 = p2[b][:, y2, x2]            # [C, N]
    dist = TEMP * des1.T @ des2        # [N, N]
    loss_b = 2*trace(dist) - sum_m lse_row[m] - sum_n lse_col[n]
    loss   = -(sum_b loss_b) / (B*N)

Key identities used on-device:
    trace(dist)   = TEMP * <des1, des2>_Frobenius  (elementwise, no matmul)
    lse (no max-subtraction) is safe: |dist| <~ 20, exp fits fp32/bf16 range.

Sharding: data-parallel over the batch dim, one batch element per
NeuronCore (B == 8 == n_cores).  The host gathers descriptors with the
int32 index arrays (pure data movement), casts to bf16, runs the SPMD
program, and averages the 8 per-core partial sums.

Per-core engine assignment:
    PE : dist tiles (bf16 inputs, fp32 PSUM accumulate over C=2x128)
         + ones-matmul partition reductions at the end
    ACT: exp(TEMP*dist) PSUM->SBUF(bf16), accum_out = row sums (free!)
         + final Ln (with accum_out again for the sum of logs)
    DVE: column-sum accumulation in bf16 (2x mode) + diag term via
         scalar_tensor_tensor's accum_out

Pipeline: each m-tile's 3540 dist columns live in three PSUM regions
(1024+1024+1492 fp32 = 2+2+3 banks).  PE refills a region only after
ACT finished exp-ing it (WAR tracked at byte granularity); each refill
fits inside ACT's work on the other regions, so the steady state is
ACT-bound at ~3.9us per m-tile.  Tile 0 is exp-ed in 512-wide
sub-chunks so ACT starts as soon as the first two matmuls finish.
"""

import numpy as np
import ml_dtypes

B = 8
C = 256
N = 3540
TEMP = 0.2
KP = 128           # contraction chunk (partition dim)
NK = C // KP       # 2
MT = 128           # rows per m-tile
N_MTILES = (N + MT - 1) // MT   # 28 (last tile has 84 rows)
REGIONS = [(0, 1024), (1024, 1024), (2048, N - 2048)]   # 2+2+3 PSUM banks
MM_N = 512         # max moving free dim per matmul
HEAD = 512         # fast-start column split

_prog_cache = {}


def _mm_chunks(width):
    out = []
    off = 0
    while off < width:
        w = min(MM_N, width - off)
        out.append((off, w))
        off += w
    return out


def _build_program():
    import contextlib
    import concourse.bacc as bacc
    import concourse.tile as tile
    from concourse import mybir

    dt = mybir.dt
    f32 = dt.float32
    bf16 = dt.bfloat16
    Exp = mybir.ActivationFunctionType.Exp
    Ln = mybir.ActivationFunctionType.Ln
    MULT = mybir.AluOpType.mult
    AX = mybir.AxisListType.X

    nc = bacc.Bacc(
        "TRN2", target_bir_lowering=False, debug=False, num_devices=B)
    d1 = nc.dram_tensor("d1", [C, N], bf16, kind="ExternalInput")
    d2 = nc.dram_tensor("d2", [C, N], bf16, kind="ExternalInput")
    out = nc.dram_tensor("out", [1, 3], f32, kind="ExternalOutput")

    with tile.TileContext(nc) as tc:
        with (
            tc.tile_pool(name="persist", bufs=1) as persist,
            tc.tile_pool(name="etiles", bufs=2) as etiles,
            tc.tile_pool(name="small", bufs=1) as small,
        ):
            # ---- load descriptors (bf16, [128, N] per C-chunk) ----
            # Split + ordered so tile 0's operands land first, spread over
            # two HWDGE queues (sync + scalar).
            d1_sb = [persist.tile([KP, N], bf16, tag=f"d1_{k}", name=f"d1_{k}")
                     for k in range(NK)]
            d2_sb = [persist.tile([KP, N], bf16, tag=f"d2_{k}", name=f"d2_{k}")
                     for k in range(NK)]
            for k in range(NK):   # lhsT columns for m-tiles 0..3
                nc.sync.dma_start(out=d1_sb[k][:, 0:HEAD],
                                  in_=d1[k * KP:(k + 1) * KP, 0:HEAD])
            for k in range(NK):   # first 512 dist columns
                nc.scalar.dma_start(out=d2_sb[k][:, 0:HEAD],
                                    in_=d2[k * KP:(k + 1) * KP, 0:HEAD])
            for k in range(NK):   # rest of regions 0+1
                nc.scalar.dma_start(out=d2_sb[k][:, HEAD:2048],
                                    in_=d2[k * KP:(k + 1) * KP, HEAD:2048])
            for k in range(NK):   # region 2
                nc.sync.dma_start(out=d2_sb[k][:, 2048:N],
                                  in_=d2[k * KP:(k + 1) * KP, 2048:N])
            for k in range(NK):   # rest of des1
                nc.scalar.dma_start(out=d1_sb[k][:, HEAD:N],
                                    in_=d1[k * KP:(k + 1) * KP, HEAD:N])

            colacc = persist.tile([MT, N], bf16, tag="colacc", name="colacc")
            nc.vector.memset(colacc, 0.0)

            # rsparts[:, r*N_MTILES + t] = rowsum of exp over region r of
            # m-tile t.  1/3-init: rows of the last (84-row) m-tile that do
            # not exist sum to 1.0 -> Ln contributes 0.  Tile 0's rowsum is
            # assembled separately (rs0) and overwrites column 0.
            rsparts = small.tile([MT, 3 * N_MTILES], f32, tag="rsparts",
                                 name="rsparts")
            nc.vector.memset(rsparts, 1.0 / 3.0)
            rs0 = small.tile([MT, 7], f32, tag="rs0", name="rs0")

            ones_bf = small.tile([KP, 1], bf16, name="ones_bf")
            nc.vector.memset(ones_bf, 1.0)
            ones_f32 = small.tile([KP, 1], f32, name="ones_f32")
            nc.vector.memset(ones_f32, 1.0)

            # fin[:,0] = diag partial, fin[:,1] = sum of row-logs partial
            fin = small.tile([KP, 2], f32, tag="fin", name="fin")

            # ---- diag term: sum(des1 * des2) per partition ----
            # (tensor_tensor_reduce wedges the device; scalar_tensor_tensor
            # with accum_out is the stable fused multiply+rowsum.)
            scratch = persist.tile([KP, N], bf16, tag="scratch", name="scratch")
            diag0 = small.tile([KP, 1], f32, name="diag0")
            diag1 = small.tile([KP, 1], f32, name="diag1")
            nc.vector.scalar_tensor_tensor(
                out=scratch, in0=d1_sb[0], scalar=1.0, in1=d2_sb[0],
                op0=MULT, op1=MULT, accum_out=diag0)
            nc.vector.scalar_tensor_tensor(
                out=scratch, in0=d1_sb[1], scalar=1.0, in1=d2_sb[1],
                op0=MULT, op1=MULT, accum_out=diag1)
            nc.vector.tensor_add(fin[:, 0:1], diag0, diag1)

            # ---- main loop over m-tiles ----
            with contextlib.ExitStack() as psctx:
                pspools = [
                    psctx.enter_context(
                        tc.tile_pool(name=f"ps{r}", bufs=1, space="PSUM"))
                    for r in range(3)
                ]
                for t in range(N_MTILES):
                    m0 = t * MT
                    mp = min(MT, N - m0)
                    ps = [pspools[r].tile([MT, REGIONS[r][1]], f32,
                                          tag=f"ps{r}", name=f"ps{r}")
                          for r in range(3)]
                    # PE: regions 0+1 k-outer (weight reuse), then region 2.
                    for k in range(NK):
                        for r in (0, 1):
                            g, gw = REGIONS[r]
                            for (off, w) in _mm_chunks(gw):
                                nc.tensor.matmul(
                                    ps[r][:mp, off:off + w],
                                    lhsT=d1_sb[k][:, m0:m0 + mp],
                                    rhs=d2_sb[k][:, g + off:g + off + w],
                                    start=(k == 0), stop=(k == NK - 1))
                    g2, gw2 = REGIONS[2]
                    for k in range(NK):
                        for (off, w) in _mm_chunks(gw2):
                            nc.tensor.matmul(
                                ps[2][:mp, off:off + w],
                                lhsT=d1_sb[k][:, m0:m0 + mp],
                                rhs=d2_sb[k][:, g2 + off:g2 + off + w],
                                start=(k == 0), stop=(k == NK - 1))

                    # ACT: exp -> bf16 SBUF + rowsum accum; DVE: colacc add.
                    # Tile 0 is processed in 512-wide sub-chunks so the
                    # pipeline starts right after the first two matmuls.
                    sub = 0
                    for r in range(3):
                        g, gw = REGIONS[r]
                        e = etiles.tile([MT, gw], bf16, tag=f"e{r}",
                                        name=f"e{r}")
                        if t == 0:
                            for (off, w) in _mm_chunks(gw):
                                nc.scalar.activation(
                                    out=e[:mp, off:off + w],
                                    in_=ps[r][:mp, off:off + w], func=Exp,
                                    scale=TEMP,
                                    accum_out=rs0[:mp, sub:sub + 1])
                                nc.vector.tensor_add(
                                    colacc[:mp, g + off:g + off + w],
                                    colacc[:mp, g + off:g + off + w],
                                    e[:mp, off:off + w])
                                sub += 1
                        else:
                            nc.scalar.activation(
                                out=e[:mp, :], in_=ps[r][:mp, :], func=Exp,
                                scale=TEMP,
                                accum_out=rsparts[:mp, r * N_MTILES + t:
                                                  r * N_MTILES + t + 1])
                            nc.vector.tensor_add(
                                colacc[:mp, g:g + gw],
                                colacc[:mp, g:g + gw],
                                e[:mp, :])

            # ---- finalize ----
            # rowsums; invalid rows = 1.0 -> Ln 0; tile 0 from rs0.
            rowsums = small.tile([MT, N_MTILES], f32, tag="rowsums",
                                 name="rowsums")
            nc.vector.tensor_add(
                rowsums, rsparts[:, 0:N_MTILES],
                rsparts[:, N_MTILES:2 * N_MTILES])
            nc.vector.tensor_add(
                rowsums, rowsums, rsparts[:, 2 * N_MTILES:3 * N_MTILES])
            nc.vector.reduce_sum(out=rowsums[:, 0:1], in_=rs0, axis=AX)
            rl = small.tile([MT, N_MTILES], f32, tag="rl", name="rl")
            nc.scalar.activation(out=rl, in_=rowsums, func=Ln,
                                 accum_out=fin[:, 1:2])

            with tc.tile_pool(name="psF", bufs=1, space="PSUM") as psF:
                # column sums: ones-matmuls into one 7-bank PSUM strip,
                # then a single Ln whose accum_out is sum(log(colsum)).
                csum = psF.tile([1, 3584], f32, tag="csum", name="csum")
                for (off, w) in _mm_chunks(N):
                    nc.tensor.matmul(csum[0:1, off:off + w], lhsT=ones_bf,
                                     rhs=colacc[:, off:off + w],
                                     start=True, stop=True)
                cl = small.tile([1, N], f32, tag="cl", name="cl")
                clsum = small.tile([1, 1], f32, tag="clsum", name="clsum")
                nc.scalar.activation(out=cl, in_=csum[0:1, 0:N], func=Ln,
                                     accum_out=clsum)

                # partition-reduce diag and row-log partials in one matmul
                dr_ps = psF.tile([1, 2], f32, tag="drps", name="dr_ps")
                nc.tensor.matmul(dr_ps[0:1, 0:2], lhsT=ones_f32,
                                 rhs=fin[:, 0:2], start=True, stop=True)

                outsb = small.tile([1, 3], f32, tag="outsb", name="outsb")
                nc.vector.tensor_copy(outsb[0:1, 0:2], dr_ps[0:1, 0:2])
                nc.vector.tensor_copy(outsb[0:1, 2:3], clsum)
                nc.sync.dma_start(out=out[:, :], in_=outsb)

    nc.compile()
    return nc


def _get_program():
    if "nc" not in _prog_cache:
        _prog_cache["nc"] = _build_program()
    return _prog_cache["nc"]


def kernel(**inputs) -> np.ndarray:
    from concourse.bass_utils import run_bass_kernel_spmd

    p1 = np.asarray(inputs["p1"], dtype=np.float32)
    p2 = np.asarray(inputs["p2"], dtype=np.float32)
    y1 = np.asarray(inputs["y1"]).astype(np.int64)
    x1 = np.asarray(inputs["x1"]).astype(np.int64)
    y2 = np.asarray(inputs["y2"]).astype(np.int64)
    x2 = np.asarray(inputs["x2"]).astype(np.int64)

    # Host-side gather (data movement only): [B, C, N] then bf16 cast.
    des1 = p1[:, :, y1, x1].astype(ml_dtypes.bfloat16)
    des2 = p2[:, :, y2, x2].astype(ml_dtypes.bfloat16)

    nc = _get_program()
    in_maps = [
        {"d1": np.ascontiguousarray(des1[b]), "d2": np.ascontiguousarray(des2[b])}
        for b in range(B)
    ]
    res = run_bass_kernel_spmd(nc, in_maps, list(range(B)))
    total = 0.0
    for b in range(B):
        d, r, c = (float(v) for v in np.asarray(res.results[b]["out"]).ravel())
        total += 2.0 * TEMP * d - r - c
    loss = -total / (B * N)
    return np.float32(loss)


# revision 12
# speedup vs baseline: 1.0357x; 1.0141x over previous
"""Trainium2 Bass kernel for the DescriptorLoss dual-softmax loss.

Math (per batch element b):
    des1 = p1[b][:, y1, x1]            # [C=256, N=3540]
    des2 = p2[b][:, y2, x2]            # [C, N]
    dist = TEMP * des1.T @ des2        # [N, N]
    loss_b = 2*trace(dist) - sum_m lse_row[m] - sum_n lse_col[n]
    loss   = -(sum_b loss_b) / (B*N)

Key identities used on-device:
    trace(dist)   = TEMP * <des1, des2>_Frobenius  (elementwise, no matmul)
    lse (no max-subtraction) is safe: |dist| <~ 20, exp fits fp32/bf16 range.

Sharding: data-parallel over the batch dim, one batch element per
NeuronCore (B == 8 == n_cores).  The host gathers descriptors with the
int32 index arrays (pure data movement), casts to bf16, runs the SPMD
program, and averages the 8 per-core partial sums.

Per-core engine assignment:
    PE : dist tiles (bf16 inputs, fp32 PSUM accumulate over C=2x128)
         + ones-matmul partition reductions at the end
    ACT: exp(TEMP*dist) PSUM->SBUF(bf16), accum_out = row sums (free!)
         + final Ln (with accum_out again for the sum of logs)
    DVE: column-sum accumulation in bf16 (2x mode) + diag term via
         scalar_tensor_tensor's accum_out

Pipeline: each m-tile's 3540 dist columns live in three PSUM regions
(1024+1024+1492 fp32 = 2+2+3 banks).  PE refills a region only after
ACT finished exp-ing it (WAR tracked at byte granularity); each refill
fits inside ACT's work on the other regions, so the steady state is
ACT-bound at ~3.9us per m-tile.  Tile 0 is exp-ed in 512-wide
sub-chunks so ACT starts as soon as the first two matmuls finish.
"""

import numpy as np
import ml_dtypes

B = 8
C = 256
N = 3540
TEMP = 0.2
KP = 128           # contraction chunk (partition dim)
NK = C // KP       # 2
MT = 128           # rows per m-tile
N_MTILES = (N + MT - 1) // MT   # 28 (last tile has 84 rows)
REGIONS = [(0, 1024), (1024, 1024), (2048, N - 2048)]   # 2+2+3 PSUM banks
MM_N = 512         # max moving free dim per matmul
HEAD = 512         # fast-start column split

_prog_cache = {}


def _mm_chunks(width):
    out = []
    off = 0
    while off < width:
        w = min(MM_N, width - off)
        out.append((off, w))
        off += w
    return out


def _build_program():
    import contextlib
    import concourse.bacc as bacc
    import concourse.tile as tile
    from concourse import mybir

    dt = mybir.dt
    f32 = dt.float32
    bf16 = dt.bfloat16
    Exp = mybir.ActivationFunctionType.Exp
    Ln = mybir.ActivationFunctionType.Ln
    MULT = mybir.AluOpType.mult
    AX = mybir.AxisListType.X

    nc = bacc.Bacc(
        "TRN2", target_bir_lowering=False, debug=False, num_devices=B)
    d1 = nc.dram_tensor("d1", [C, N], bf16, kind="ExternalInput")
    d2 = nc.dram_tensor("d2", [C, N], bf16, kind="ExternalInput")
    out = nc.dram_tensor("out", [1, 3], f32, kind="ExternalOutput")

    with tile.TileContext(nc) as tc:
        with (
            tc.tile_pool(name="persist", bufs=1) as persist,
            tc.tile_pool(name="etiles", bufs=2) as etiles,
            tc.tile_pool(name="small", bufs=1) as small,
        ):
            # ---- load descriptors (bf16, [128, N] per C-chunk) ----
            # Split + ordered so tile 0's operands land first, spread over
            # two HWDGE queues (sync + scalar).
            d1_sb = [persist.tile([KP, N], bf16, tag=f"d1_{k}", name=f"d1_{k}")
                     for k in range(NK)]
            d2_sb = [persist.tile([KP, N], bf16, tag=f"d2_{k}", name=f"d2_{k}")
                     for k in range(NK)]
            for k in range(NK):   # lhsT columns for m-tiles 0..3
                nc.sync.dma_start(out=d1_sb[k][:, 0:HEAD],
                                  in_=d1[k * KP:(k + 1) * KP, 0:HEAD])
            for k in range(NK):   # first 512 dist columns
                nc.scalar.dma_start(out=d2_sb[k][:, 0:HEAD],
                                    in_=d2[k * KP:(k + 1) * KP, 0:HEAD])
            for k in range(NK):   # rest of regions 0+1
                nc.scalar.dma_start(out=d2_sb[k][:, HEAD:2048],
                                    in_=d2[k * KP:(k + 1) * KP, HEAD:2048])
            for k in range(NK):   # region 2
                nc.sync.dma_start(out=d2_sb[k][:, 2048:N],
                                  in_=d2[k * KP:(k + 1) * KP, 2048:N])
            for k in range(NK):   # rest of des1
                nc.scalar.dma_start(out=d1_sb[k][:, HEAD:N],
                                    in_=d1[k * KP:(k + 1) * KP, HEAD:N])

            colacc = persist.tile([MT, N], bf16, tag="colacc", name="colacc")
            nc.vector.memset(colacc, 0.0)

            # rsparts[:, r*N_MTILES + t] = rowsum of exp over region r of
            # m-tile t.  1/3-init: rows of the last (84-row) m-tile that do
            # not exist sum to 1.0 -> Ln contributes 0.  Tile 0's rowsum is
            # assembled separately (rs0) and overwrites column 0.
            rsparts = small.tile([MT, 3 * N_MTILES], f32, tag="rsparts",
                                 name="rsparts")
            nc.vector.memset(rsparts, 1.0 / 3.0)
            rs0 = small.tile([MT, 7], f32, tag="rs0", name="rs0")

            ones_bf = small.tile([KP, 1], bf16, name="ones_bf")
            nc.vector.memset(ones_bf, 1.0)
            ones_f32 = small.tile([KP, 1], f32, name="ones_f32")
            nc.vector.memset(ones_f32, 1.0)

            # fin[:,0] = diag partial, fin[:,1] = sum of row-logs partial
            fin = small.tile([KP, 2], f32, tag="fin", name="fin")

            # ---- diag term: sum(des1 * des2) per partition ----
            # (tensor_tensor_reduce wedges the device; scalar_tensor_tensor
            # with accum_out is the stable fused multiply+rowsum.)
            scratch = persist.tile([KP, N], bf16, tag="scratch", name="scratch")
            diag0 = small.tile([KP, 1], f32, name="diag0")
            diag1 = small.tile([KP, 1], f32, name="diag1")
            nc.vector.scalar_tensor_tensor(
                out=scratch, in0=d1_sb[0], scalar=1.0, in1=d2_sb[0],
                op0=MULT, op1=MULT, accum_out=diag0)
            nc.vector.scalar_tensor_tensor(
                out=scratch, in0=d1_sb[1], scalar=1.0, in1=d2_sb[1],
                op0=MULT, op1=MULT, accum_out=diag1)
            nc.vector.tensor_add(fin[:, 0:1], diag0, diag1)

            # ---- main loop over m-tiles ----
            with contextlib.ExitStack() as psctx:
                pspools = [
                    psctx.enter_context(
                        tc.tile_pool(name=f"ps{r}", bufs=1, space="PSUM"))
                    for r in range(3)
                ]
                for t in range(N_MTILES):
                    m0 = t * MT
                    mp = min(MT, N - m0)
                    ps = [pspools[r].tile([MT, REGIONS[r][1]], f32,
                                          tag=f"ps{r}", name=f"ps{r}")
                          for r in range(3)]
                    # PE: regions 0+1 k-outer (weight reuse), then region 2.
                    # Tile 0 goes chunk-outer/k-inner instead, so each
                    # 512-chunk is exp-able right after two matmuls.
                    if t == 0:
                        for r in range(3):
                            g, gw = REGIONS[r]
                            for (off, w) in _mm_chunks(gw):
                                for k in range(NK):
                                    nc.tensor.matmul(
                                        ps[r][:mp, off:off + w],
                                        lhsT=d1_sb[k][:, m0:m0 + mp],
                                        rhs=d2_sb[k][:, g + off:g + off + w],
                                        start=(k == 0), stop=(k == NK - 1))
                    else:
                        for k in range(NK):
                            for r in (0, 1):
                                g, gw = REGIONS[r]
                                for (off, w) in _mm_chunks(gw):
                                    nc.tensor.matmul(
                                        ps[r][:mp, off:off + w],
                                        lhsT=d1_sb[k][:, m0:m0 + mp],
                                        rhs=d2_sb[k][:, g + off:g + off + w],
                                        start=(k == 0), stop=(k == NK - 1))
                        g2, gw2 = REGIONS[2]
                        for k in range(NK):
                            for (off, w) in _mm_chunks(gw2):
                                nc.tensor.matmul(
                                    ps[2][:mp, off:off + w],
                                    lhsT=d1_sb[k][:, m0:m0 + mp],
                                    rhs=d2_sb[k][:, g2 + off:g2 + off + w],
                                    start=(k == 0), stop=(k == NK - 1))

                    # ACT: exp -> bf16 SBUF + rowsum accum; DVE: colacc add.
                    # Tile 0 is processed in 512-wide sub-chunks so the
                    # pipeline starts right after the first two matmuls.
                    sub = 0
                    for r in range(3):
                        g, gw = REGIONS[r]
                        e = etiles.tile([MT, gw], bf16, tag=f"e{r}",
                                        name=f"e{r}")
                        if t == 0:
                            for (off, w) in _mm_chunks(gw):
                                nc.scalar.activation(
                                    out=e[:mp, off:off + w],
                                    in_=ps[r][:mp, off:off + w], func=Exp,
                                    scale=TEMP,
                                    accum_out=rs0[:mp, sub:sub + 1])
                                nc.vector.tensor_add(
                                    colacc[:mp, g + off:g + off + w],
                                    colacc[:mp, g + off:g + off + w],
                                    e[:mp, off:off + w])
                                sub += 1
                        else:
                            nc.scalar.activation(
                                out=e[:mp, :], in_=ps[r][:mp, :], func=Exp,
                                scale=TEMP,
                                accum_out=rsparts[:mp, r * N_MTILES + t:
                                                  r * N_MTILES + t + 1])
                            nc.vector.tensor_add(
                                colacc[:mp, g:g + gw],
                                colacc[:mp, g:g + gw],
                                e[:mp, :])

            # ---- finalize ----
            # rowsums; invalid rows = 1.0 -> Ln 0; tile 0 from rs0.
            rowsums = small.tile([MT, N_MTILES], f32, tag="rowsums",
                                 name="rowsums")
            nc.vector.tensor_add(
                rowsums, rsparts[:, 0:N_MTILES],
                rsparts[:, N_MTILES:2 * N_MTILES])
            nc.vector.tensor_add(
                rowsums, rowsums, rsparts[:, 2 * N_MTILES:3 * N_MTILES])
            nc.vector.reduce_sum(out=rowsums[:, 0:1], in_=rs0, axis=AX)
            rl = small.tile([MT, N_MTILES], f32, tag="rl", name="rl")
            nc.scalar.activation(out=rl, in_=rowsums, func=Ln,
                                 accum_out=fin[:, 1:2])

            with tc.tile_pool(name="psF", bufs=1, space="PSUM") as psF:
                # column sums: ones-matmuls into one 7-bank PSUM strip,
                # then a single Ln whose accum_out is sum(log(colsum)).
                csum = psF.tile([1, 3584], f32, tag="csum", name="csum")
                for (off, w) in _mm_chunks(N):
                    nc.tensor.matmul(csum[0:1, off:off + w], lhsT=ones_bf,
                                     rhs=colacc[:, off:off + w],
                                     start=True, stop=True)
                cl = small.tile([1, N], f32, tag="cl", name="cl")
                clsum = small.tile([1, 1], f32, tag="clsum", name="clsum")
                nc.scalar.activation(out=cl, in_=csum[0:1, 0:N], func=Ln,
                                     accum_out=clsum)

                # partition-reduce diag and row-log partials in one matmul
                dr_ps = psF.tile([1, 2], f32, tag="drps", name="dr_ps")
                nc.tensor.matmul(dr_ps[0:1, 0:2], lhsT=ones_f32,
                                 rhs=fin[:, 0:2], start=True, stop=True)

                outsb = small.tile([1, 3], f32, tag="outsb", name="outsb")
                nc.vector.tensor_copy(outsb[0:1, 0:2], dr_ps[0:1, 0:2])
                nc.vector.tensor_copy(outsb[0:1, 2:3], clsum)
                nc.sync.dma_start(out=out[:, :], in_=outsb)

    nc.compile()
    return nc


def _get_program():
    if "nc" not in _prog_cache:
        _prog_cache["nc"] = _build_program()
    return _prog_cache["nc"]


def kernel(**inputs) -> np.ndarray:
    from concourse.bass_utils import run_bass_kernel_spmd

    p1 = np.asarray(inputs["p1"], dtype=np.float32)
    p2 = np.asarray(inputs["p2"], dtype=np.float32)
    y1 = np.asarray(inputs["y1"]).astype(np.int64)
    x1 = np.asarray(inputs["x1"]).astype(np.int64)
    y2 = np.asarray(inputs["y2"]).astype(np.int64)
    x2 = np.asarray(inputs["x2"]).astype(np.int64)

    # Host-side gather (data movement only): [B, C, N] then bf16 cast.
    des1 = p1[:, :, y1, x1].astype(ml_dtypes.bfloat16)
    des2 = p2[:, :, y2, x2].astype(ml_dtypes.bfloat16)

    nc = _get_program()
    in_maps = [
        {"d1": np.ascontiguousarray(des1[b]), "d2": np.ascontiguousarray(des2[b])}
        for b in range(B)
    ]
    res = run_bass_kernel_spmd(nc, in_maps, list(range(B)))
    total = 0.0
    for b in range(B):
        d, r, c = (float(v) for v in np.asarray(res.results[b]["out"]).ravel())
        total += 2.0 * TEMP * d - r - c
    loss = -total / (B * N)
    return np.float32(loss)
